# revision 1
# baseline (speedup 1.0000x reference)
"""GCN (6-layer, symmetric-normalized) on 8 Trainium2 NeuronCores.

Strategy (graph/data parallel, per sharding hint):
  - Nodes are binned onto 8 cores x 49 blocks x 128 slots (degree-balanced
    snake deal).  Each core owns the destination nodes of its slots.
  - Per conv layer l=1..5 (F_out < F_in): each core computes
    g = dis * (h @ W) for its rows (PE matmul), AllGathers g into a
    replicated table G, then aggregates incoming messages for its
    destinations: rows of G are fetched with dma_gather (SWDGE custom
    gather) in dst-grouped chunks of 128 and segment-summed with one-hot
    selection matmuls accumulating in PSUM.  out = dis*(sum + g_self) + b,
    ReLU.
  - Layer 6 aggregates first (F=128 < 10's matmul width is tiny):
    z = dis*(agg(g5) + g5_self), then out = log_softmax(z @ W6 + b6).
  - deg = 1 + in-degree (self loops), dis = 1/sqrt(deg) computed on device.

The gather index space is the AllGather output table [50176, F]; int16
gather indices are bucketed (< 32768 / >= 32768) with two table bases.
Padding slots point at designated always-zero rows (fake node slots).
"""
import sys
sys.path.insert(0, "/opt/trn_rl_repo")

import numpy as np

import concourse.bacc as bacc
import concourse.bass as bass
import concourse.mybir as mybir
import concourse.tile as tile
from concourse.bass_utils import run_bass_kernel_spmd
from concourse.library_config import mlp
from concourse.masks import make_identity

F32 = mybir.dt.float32
BF16 = mybir.dt.bfloat16
I32 = mybir.dt.int32
I16 = mybir.dt.int16

N = 50000
E = 800000
NCORES = 8
NB = 49                    # blocks per core
SLOTS = NB * 128           # 6272 node slots per core
NROWS = NCORES * SLOTS     # 50176 rows in the gathered tables
BSPLIT = 32768             # int16 gather bucket boundary
DIMS = [767, 640, 512, 384, 256, 128, 10]
# aggregation feature width per layer (layers 1..5 aggregate post-transform,
# layer 6 aggregates pre-transform)
AGGF = [640, 512, 384, 256, 128, 128]


# ----------------------------------------------------------------- host prep

def _assign_nodes(deg_in):
    """Snake-deal nodes (desc by in-degree) onto (core, blk, slot)."""
    order = np.argsort(-deg_in, kind="stable")
    nblocks = NCORES * NB
    k = np.arange(N)
    rnd = k // nblocks
    col = k % nblocks
    block = np.where(rnd % 2 == 0, col, nblocks - 1 - col)
    slot = rnd
    core = block % NCORES
    blk = block // NCORES
    grow = np.empty(N, np.int64)
    grow[order] = core * SLOTS + blk * 128 + slot
    return grow


def _prepare(inputs):
    x = np.asarray(inputs["x"], np.float32)
    ei = np.asarray(inputs["edge_index"])
    src, dst = ei[0].astype(np.int64), ei[1].astype(np.int64)

    deg_in = np.bincount(dst, minlength=N).astype(np.int64)
    grow = _assign_nodes(deg_in)

    # fake (unused) slots; designate two as the all-zero pad rows
    used = np.zeros(NROWS, bool)
    used[grow] = True
    fakes = np.nonzero(~used)[0]
    fa = fakes[fakes < BSPLIT]
    fb = fakes[fakes >= BSPLIT]
    assert len(fa) > 0 and len(fb) > 0, (len(fa), len(fb))
    ZA, ZB = int(fa[0]), int(fb[0])

    gs = grow[src]
    gd = grow[dst]
    cd = gd // SLOTS
    bd = (gd % SLOTS) // 128
    sd = gd % 128
    bucket = (gs >= BSPLIT).astype(np.int64)

    gid = (cd * NB + bd) * 2 + bucket
    order = np.lexsort((gs, gid))
    gid_s = gid[order]
    gs_s = gs[order]
    sd_s = sd[order]
    bucket_s = bucket[order]

    counts = np.bincount(gid, minlength=NCORES * NB * 2)
    nA = counts[0::2].reshape(NCORES, NB)
    nB = counts[1::2].reshape(NCORES, NB)
    CA = int(-(-nA.max() // 128))
    CB = int(-(-nB.max() // 128))
    CH = CA + CB
    TOTCH = NB * CH

    grp_start = np.concatenate([[0], np.cumsum(counts)])[:-1]
    pos_in_grp = np.arange(E) - grp_start[gid_s]
    pos = pos_in_grp + np.where(bucket_s == 0, 0, CA * 128)
    core_s = gid_s // (2 * NB)
    blk_s = (gid_s // 2) % NB
    flat = blk_s * (CH * 128) + pos        # position within the core's stream

    # per-core padded index/slot arrays
    padrow = np.empty(CH * 128, np.int64)
    padrow[: CA * 128] = ZA
    padrow[CA * 128:] = ZB - BSPLIT
    idxvals = np.tile(padrow, (NCORES, NB))          # [NCORES, NB*CH*128]
    slotvals = np.full((NCORES, NB * CH * 128), 999.0, np.float32)
    relidx = np.where(bucket_s == 0, gs_s, gs_s - BSPLIT)
    idxvals[core_s, flat] = relidx
    slotvals[core_s, flat] = sd_s

    in_maps = []
    icnt_all = np.zeros((NCORES, NB, 128), np.float32)
    np.add.at(icnt_all, (grow // SLOTS, (grow % SLOTS) // 128, grow % 128),
              deg_in.astype(np.float32))
    xl_all = np.zeros((NCORES, SLOTS, DIMS[0]), np.float32)
    xl_all[grow // SLOTS, grow % SLOTS] = x

    wb = {}
    for l in range(1, 7):
        fin, fout = DIMS[l - 1], DIMS[l]
        W = np.asarray(inputs[f"W{l}"], np.float32)
        b = np.asarray(inputs[f"b{l}"], np.float32)
        K = -(-fin // 128)
        Wr = np.zeros((128, K * fout), np.float32)
        for k in range(K):
            r0, r1 = k * 128, min((k + 1) * 128, fin)
            Wr[: r1 - r0, k * fout:(k * fout) + fout] = W[r0:r1]
        wb[f"W{l}"] = Wr
        wb[f"b{l}"] = np.broadcast_to(b, (128, fout)).copy()

    for c in range(NCORES):
        iv = idxvals[c].reshape(TOTCH, 128).astype(np.int16)
        idx16 = np.tile(
            iv.reshape(TOTCH, 8, 16).transpose(2, 0, 1).reshape(16, TOTCH * 8),
            (8, 1)).copy()
        slots = slotvals[c].reshape(TOTCH, 128).T.copy()
        m = {
            "x": xl_all[c],
            "idx16": idx16,
            "slots": slots,
            "icnt": icnt_all[c].T.copy(),   # [slot, blk]
        }
        m.update(wb)
        in_maps.append(m)

    meta = {"CA": CA, "CB": CB, "grow": grow}
    return in_maps, meta


# ------------------------------------------------------------ device program

def _build(CA, CB):
    import os
    skip_ag = bool(int(os.environ.get("GCN_SKIP_AG", "0")))
    skip_gather = bool(int(os.environ.get("GCN_SKIP_GATHER", "0")))
    skip_smm = bool(int(os.environ.get("GCN_SKIP_SMM", "0")))
    use_bf16 = not bool(int(os.environ.get("GCN_FP32", "0")))
    subg = bool(int(os.environ.get("GCN_SUBG", "1")))
    fuse = bool(int(os.environ.get("GCN_FUSE", "0")))
    bias_zero = bool(int(os.environ.get("GCN_BIAS_ZERO", "0")))
    mbufs = int(os.environ.get("GCN_MBUFS", "2"))
    spkt = bool(int(os.environ.get("GCN_SPKT", "0")))
    nq = int(os.environ.get("GCN_NQ", "4"))
    MDT = BF16 if use_bf16 else F32
    CH = CA + CB
    TOTCH = NB * CH
    nc = bacc.Bacc("TRN2", target_bir_lowering=False, debug=False,
                   num_devices=NCORES, num_swdge_queues=4)

    x_in = nc.dram_tensor("x", [SLOTS, DIMS[0]], F32, kind="ExternalInput")
    idx16_in = nc.dram_tensor("idx16", [128, TOTCH * 8], I16,
                              kind="ExternalInput")
    slots_in = nc.dram_tensor("slots", [128, TOTCH], F32, kind="ExternalInput")
    icnt_in = nc.dram_tensor("icnt", [128, NB], F32, kind="ExternalInput")
    W_in, b_in = {}, {}
    for l in range(1, 7):
        fin, fout = DIMS[l - 1], DIMS[l]
        K = -(-fin // 128)
        W_in[l] = nc.dram_tensor(f"W{l}", [128, K * fout], F32,
                                 kind="ExternalInput")
        b_in[l] = nc.dram_tensor(f"b{l}", [128, fout], F32,
                                 kind="ExternalInput")
    out_t = nc.dram_tensor("out", [SLOTS, DIMS[6]], F32, kind="ExternalOutput")

    rg = [list(range(NCORES))]
    qn = [0]

    def next_q():
        qn[0] = (qn[0] + 1) % nq
        return qn[0]

    with tile.TileContext(nc) as tc:
        with (
            tc.tile_pool(name="const", bufs=1) as constp,
            tc.tile_pool(name="wpool", bufs=2) as wpool,
            tc.tile_pool(name="msg", bufs=2) as msgp,
            tc.tile_pool(name="sel", bufs=4) as selp,
            tc.tile_pool(name="work", bufs=2) as workp,
            tc.tile_pool(name="tsp", bufs=3) as tspp,
            tc.tile_pool(name="ps", bufs=1, space="PSUM") as psp,
            tc.tile_pool(name="dram", bufs=1, space="DRAM") as dramp,
        ):
            nc.gpsimd.load_library(mlp)

            idx16_sb = constp.tile([128, TOTCH * 8], I16)
            nc.sync.dma_start(out=idx16_sb[:], in_=idx16_in[:])
            slots_sb = constp.tile([128, TOTCH], F32)
            nc.sync.dma_start(out=slots_sb[:], in_=slots_in[:])

            iota = constp.tile([128, 128], F32)
            nc.gpsimd.iota(iota[:], [[1, 128]], channel_multiplier=0,
                           allow_small_or_imprecise_dtypes=True)
            iota4 = constp.tile([128, 4, 128], F32)
            nc.gpsimd.iota(iota4[:], [[0, 4], [1, 128]], channel_multiplier=0,
                           allow_small_or_imprecise_dtypes=True)
            ident = constp.tile([128, 128], F32)
            make_identity(nc, ident[:])

            dis = constp.tile([128, NB], F32)
            nc.sync.dma_start(out=dis[:], in_=icnt_in[:])
            nc.vector.tensor_scalar_add(dis[:], dis[:], 1.0)
            nc.scalar.sqrt(dis[:], dis[:])
            nc.vector.reciprocal(dis[:], dis[:])

            # DRAM intermediates
            G = {l: dramp.tile([NROWS, AGGF[l - 1]], MDT, tag=f"G{l}",
                               name=f"G{l}", addr_space="Shared")
                 for l in range(1, 7)}
            gin = {l: dramp.tile([SLOTS, AGGF[l - 1]], MDT, tag=f"gin{l}",
                                 name=f"gin{l}") for l in range(1, 7)}
            h = {l: dramp.tile([SLOTS, DIMS[l]], F32, tag=f"h{l}",
                               name=f"h{l}") for l in range(1, 6)}

            Wts = {}
            for l in range(1, 7):
                fin, fout = DIMS[l - 1], DIMS[l]
                K = -(-fin // 128)
                Wts[l] = wpool.tile([128, K * fout], F32, tag=f"W{l}",
                                    bufs=1, name=f"Wt{l}")
                nc.sync.dma_start(out=Wts[l][:], in_=W_in[l][:])

            def col_groups(f):
                return [(0, min(f, 512))] + ([(512, f)] if f > 512 else [])

            def transform_tile(l, i, hs, fin):
                """hs = dis*h tile [128, fin] -> gin_l tile (dis*h @ W_l)."""
                fout = DIMS[l]
                K = -(-fin // 128)
                Wt = Wts[l]
                gps = psp.tile([128, fout], F32, tag="gps", bufs=1,
                               name="gps")
                hsTs = []
                for k in range(K):
                    ck = min(128, fin - k * 128)
                    tp = psp.tile([128, 128], F32, tag="tps", bufs=2,
                                  name="tp")
                    nc.tensor.transpose(out=tp[:ck, :],
                                        in_=hs[:, k * 128:k * 128 + ck],
                                        identity=ident[:])
                    hsT = tspp.tile([128, 128], F32, tag="hsT", bufs=7,
                                    name="hsT")
                    nc.scalar.activation(
                        hsT[:ck, :], tp[:ck, :],
                        mybir.ActivationFunctionType.Copy)
                    hsTs.append((hsT, ck))
                for (c0, c1) in col_groups(fout):
                    for k in range(K):
                        hsT, ck = hsTs[k]
                        nc.tensor.matmul(
                            out=gps[:, c0:c1],
                            lhsT=hsT[:ck, :],
                            rhs=Wt[:ck, k * fout + c0:k * fout + c1],
                            start=(k == 0), stop=(k == K - 1))
                gsb = workp.tile([128, fout], MDT, tag="gsb", name="gsb")
                nc.scalar.activation(gsb[:], gps[:],
                                     mybir.ActivationFunctionType.Copy)
                nc.sync.dma_start(out=gin[l][i * 128:(i + 1) * 128, :],
                                  in_=gsb[:])

            def phase_m(l):
                """h_{l-1} -> gin_l = dis * (h @ W_l)  (l = 1..5)."""
                fin = DIMS[l - 1]
                src = x_in if l == 1 else h[l - 1]
                for i in range(NB):
                    hin = workp.tile([128, fin], F32, tag="hin")
                    nc.sync.dma_start(out=hin[:],
                                      in_=src[i * 128:(i + 1) * 128, :])
                    hs = workp.tile([128, fin], F32, tag="hs")
                    nc.vector.tensor_scalar_mul(hs[:], hin[:],
                                                dis[:, i:i + 1])
                    transform_tile(l, i, hs, fin)

            def phase_m6():
                """gin_6 = dis * h5."""
                for i in range(NB):
                    hin = workp.tile([128, 128], F32, tag="hin")
                    nc.sync.dma_start(out=hin[:],
                                      in_=h[5][i * 128:(i + 1) * 128, :])
                    gsb = workp.tile([128, 128], MDT, tag="gsb")
                    nc.vector.tensor_scalar_mul(gsb[:], hin[:],
                                                dis[:, i:i + 1])
                    nc.sync.dma_start(out=gin[6][i * 128:(i + 1) * 128, :],
                                      in_=gsb[:])

            def allgather(l):
                if skip_ag:
                    return
                nc.gpsimd.collective_compute(
                    "AllGather", mybir.AluOpType.bypass,
                    replica_groups=rg,
                    ins=[gin[l].opt()],
                    outs=[G[l].opt()],
                )

            def aggregate(l, finish):
                """Gather + segment-sum block by block; `finish(b, sps)`
                consumes the per-block PSUM sum."""
                f = AGGF[l - 1]
                Gt = G[l]
                CA1 = CA if not subg else (CA + 1) // 2
                CA2 = CA - CA1
                for b in range(NB):
                    c0 = b * CH
                    mtA = msgp.tile([128, CA1, f], MDT, tag="mtA", bufs=mbufs)
                    mtA2 = (msgp.tile([128, CA2, f], MDT, tag="mtA2",
                                      name="mtA2", bufs=mbufs)
                            if CA2 else None)
                    mtB = msgp.tile([128, CB, f], MDT, tag="mtB", bufs=mbufs)
                    if not skip_gather:
                        nc.gpsimd.dma_gather(
                            mtA[:], Gt[:BSPLIT, :],
                            idx16_sb[:, c0 * 8:(c0 + CA1) * 8],
                            CA1 * 128, CA1 * 128, f,
                            single_packet=spkt, queue_num=next_q())
                        if CA2:
                            nc.gpsimd.dma_gather(
                                mtA2[:], Gt[:BSPLIT, :],
                                idx16_sb[:, (c0 + CA1) * 8:(c0 + CA) * 8],
                                CA2 * 128, CA2 * 128, f,
                                single_packet=spkt, queue_num=next_q())
                        nc.gpsimd.dma_gather(
                            mtB[:], Gt[BSPLIT:, :],
                            idx16_sb[:, (c0 + CA) * 8:(c0 + CH) * 8],
                            CB * 128, CB * 128, f,
                            single_packet=spkt, queue_num=next_q())
                    sps = psp.tile([128, f], F32, tag="sps", bufs=2)
                    Ss = []
                    for cb in range(0, CH, 4):
                        bw = min(4, CH - cb)
                        S4 = selp.tile([128, 4, 128], MDT, tag="S4",
                                       bufs=2 * (CH + 3) // 4 + 2, name="S4")
                        nc.vector.tensor_tensor(
                            out=S4[:, :bw, :],
                            in0=slots_sb[:, c0 + cb:c0 + cb + bw]
                                .to_broadcast([128, bw, 128]),
                            in1=iota4[:, :bw, :],
                            op=mybir.AluOpType.is_equal)
                        for j in range(bw):
                            Ss.append(S4[:, j, :])
                    for (g0, g1) in col_groups(f):
                        for c in range((CH if not skip_smm else 1)):
                            mt = (mtA[:, c, :] if c < CA1 else
                                  mtA2[:, c - CA1, :] if c < CA else
                                  mtB[:, c - CA, :])
                            nc.tensor.matmul(
                                out=sps[:, g0:g1], lhsT=Ss[c],
                                rhs=mt[:, g0:g1],
                                start=(c == 0),
                                stop=(c == (CH if not skip_smm else 1) - 1))
                    finish(b, sps)

            def phase_a(l):
                f = AGGF[l - 1]
                bt = wpool.tile([128, f], F32, tag="b")
                nc.sync.dma_start(out=bt[:], in_=b_in[l][:])

                def finish(b, sps):
                    gself = workp.tile([128, f], MDT, tag="gself")
                    nc.sync.dma_start(out=gself[:],
                                      in_=gin[l][b * 128:(b + 1) * 128, :])
                    ssb = workp.tile([128, f], F32, tag="ssb")
                    nc.vector.tensor_tensor(out=ssb[:], in0=sps[:],
                                            in1=gself[:],
                                            op=mybir.AluOpType.add)
                    nc.vector.tensor_scalar_mul(ssb[:], ssb[:],
                                                dis[:, b:b + 1])
                    if not bias_zero:
                        nc.vector.tensor_tensor(out=ssb[:], in0=ssb[:],
                                                in1=bt[:],
                                                op=mybir.AluOpType.add)
                    hout = workp.tile([128, f], F32, tag="hout")
                    nc.scalar.activation(hout[:], ssb[:],
                                         mybir.ActivationFunctionType.Relu)
                    if not fuse:
                        nc.sync.dma_start(out=h[l][b * 128:(b + 1) * 128, :],
                                          in_=hout[:])
                        return
                    # fused next-layer transform straight from SBUF
                    hs = workp.tile([128, f], F32, tag="hs")
                    nc.vector.tensor_scalar_mul(hs[:], hout[:],
                                                dis[:, b:b + 1])
                    if l < 5:
                        transform_tile(l + 1, b, hs, f)
                    else:
                        # layer 6 aggregates dis*h5 directly
                        g6 = workp.tile([128, 128], MDT, tag="gsb")
                        nc.vector.tensor_copy(g6[:], hs[:])
                        nc.sync.dma_start(
                            out=gin[6][b * 128:(b + 1) * 128, :], in_=g6[:])

                aggregate(l, finish)

            def phase_a6():
                W6 = Wts[6]
                b6 = wpool.tile([128, 10], F32, tag="b")
                nc.sync.dma_start(out=b6[:], in_=b_in[6][:])

                def finish(b, sps):
                    gself = workp.tile([128, 128], MDT, tag="gself")
                    nc.sync.dma_start(out=gself[:],
                                      in_=gin[6][b * 128:(b + 1) * 128, :])
                    z = workp.tile([128, 128], F32, tag="ssb")
                    nc.vector.tensor_tensor(out=z[:], in0=sps[:], in1=gself[:],
                                            op=mybir.AluOpType.add)
                    nc.vector.tensor_scalar_mul(z[:], z[:], dis[:, b:b + 1])
                    tp = psp.tile([128, 128], F32, tag="tps", bufs=2)
                    nc.tensor.transpose(out=tp[:], in_=z[:], identity=ident[:])
                    zT = tspp.tile([128, 128], F32, tag="hsT", bufs=7)
                    nc.scalar.activation(zT[:], tp[:],
                                         mybir.ActivationFunctionType.Copy)
                    ops = psp.tile([128, 128], F32, tag="tps", bufs=2)
                    nc.tensor.matmul(out=ops[:, :10], lhsT=zT[:],
                                     rhs=W6[:, :10], start=True, stop=True)
                    z10 = workp.tile([128, 10], F32, tag="z10")
                    nc.vector.tensor_tensor(out=z10[:], in0=ops[:, :10],
                                            in1=b6[:, :10],
                                            op=mybir.AluOpType.add)
                    mx = workp.tile([128, 1], F32, tag="mx")
                    nc.vector.tensor_reduce(mx[:], z10[:],
                                            mybir.AxisListType.X,
                                            mybir.AluOpType.max)
                    nmx = workp.tile([128, 1], F32, tag="nmx")
                    nc.vector.tensor_scalar_mul(nmx[:], mx[:], -1.0)
                    esum = workp.tile([128, 1], F32, tag="esum")
                    etile = workp.tile([128, 10], F32, tag="etile")
                    nc.scalar.activation(etile[:], z10[:],
                                         mybir.ActivationFunctionType.Exp,
                                         bias=nmx[:, :1], accum_out=esum[:])
                    lse = workp.tile([128, 1], F32, tag="lse")
                    nc.scalar.activation(lse[:], esum[:],
                                         mybir.ActivationFunctionType.Ln)
                    nc.vector.tensor_tensor(out=lse[:], in0=lse[:],
                                            in1=mx[:],
                                            op=mybir.AluOpType.add)
                    res = workp.tile([128, 10], F32, tag="res")
                    nc.vector.tensor_scalar(res[:], z10[:], lse[:, :1], None,
                                            mybir.AluOpType.subtract)
                    nc.sync.dma_start(out=out_t[b * 128:(b + 1) * 128, :],
                                      in_=res[:])

                aggregate(6, finish)

            if fuse:
                phase_m(1)
                for l in range(1, 6):
                    allgather(l)
                    phase_a(l)
                allgather(6)
                phase_a6()
            else:
                for l in range(1, 6):
                    phase_m(l)
                    allgather(l)
                    phase_a(l)
                phase_m6()
                allgather(6)
                phase_a6()

    nc.compile()
    return nc


# ------------------------------------------------------------------- driver

_CACHE = {}


def prepare_and_build(inputs):
    import os
    in_maps, meta = _prepare(inputs)
    if all(not np.any(np.asarray(inputs[f"b{l}"])) for l in range(1, 7)):
        os.environ.setdefault("GCN_BIAS_ZERO", "1")
    import os
    key = (meta["CA"], meta["CB"], os.environ.get("GCN_SKIP_AG"),
           os.environ.get("GCN_SKIP_GATHER"), os.environ.get("GCN_SKIP_SMM"),
           os.environ.get("GCN_FP32"), os.environ.get("GCN_SUBG"),
           os.environ.get("GCN_MBUFS"), os.environ.get("GCN_FUSE"),
           os.environ.get("GCN_BIAS_ZERO"), os.environ.get("GCN_SPKT"),
           os.environ.get("GCN_NQ"))
    if key not in _CACHE:
        _CACHE[key] = _build(meta["CA"], meta["CB"])
    return _CACHE[key], in_maps, meta


def unshard(results, meta):
    grow = meta["grow"]
    full = np.empty((N, DIMS[6]), np.float32)
    outs = np.stack([results[c]["out"] for c in range(NCORES)])
    full[:] = outs[grow // SLOTS, grow % SLOTS]
    return full


def _make_runner(nc):
    """Cached PJRT executable for repeated runs (same path
    run_bass_kernel_spmd takes under axon, minus per-call re-tracing)."""
    import jax
    from jax.sharding import Mesh, PartitionSpec
    from jax.experimental.shard_map import shard_map
    from concourse import bass2jax
    from concourse.bass2jax import _bass_exec_p, install_neuronx_cc_hook

    install_neuronx_cc_hook()
    partition_name = (nc.partition_id_tensor.name
                      if nc.partition_id_tensor else None)
    in_names, out_names, out_avals, zero_outs = [], [], [], []
    for alloc in nc.m.functions[0].allocations:
        if not isinstance(alloc, mybir.MemoryLocationSet):
            continue
        name = alloc.memorylocations[0].name
        if alloc.kind == "ExternalInput":
            if name != partition_name:
                in_names.append(name)
        elif alloc.kind == "ExternalOutput":
            out_names.append(name)
            shape = tuple(alloc.tensor_shape)
            dtype = mybir.dt.np(alloc.dtype)
            out_avals.append(jax.core.ShapedArray(shape, dtype))
            zero_outs.append(np.zeros(shape, dtype))
    n_params = len(in_names)
    all_in_names = list(in_names) + list(out_names)
    if partition_name is not None:
        all_in_names.append(partition_name)

    def _body(*args):
        operands = list(args)
        if partition_name is not None:
            operands.append(bass2jax.partition_id_tensor())
        return tuple(_bass_exec_p.bind(
            *operands,
            out_avals=tuple(out_avals),
            in_names=tuple(all_in_names),
            out_names=tuple(out_names),
            lowering_input_output_aliases=(),
            sim_require_finite=True,
            sim_require_nnan=True,
            nc=nc,
        ))

    devices = jax.devices()[:NCORES]
    mesh = Mesh(np.asarray(devices), ("core",))
    n_outs = len(out_avals)
    fn = jax.jit(
        shard_map(_body, mesh=mesh,
                  in_specs=(PartitionSpec("core"),) * (n_params + n_outs),
                  out_specs=(PartitionSpec("core"),) * n_outs,
                  check_rep=False),
        keep_unused=True)

    def run(in_maps):
        concat_in = [
            np.concatenate([np.asarray(in_maps[c][name])
                            for c in range(NCORES)], axis=0)
            for name in in_names
        ]
        concat_zeros = [
            np.zeros((NCORES * z.shape[0], *z.shape[1:]), z.dtype)
            for z in zero_outs
        ]
        out_arrs = fn(*concat_in, *concat_zeros)
        jax.block_until_ready(out_arrs)
        return [
            {name: np.asarray(out_arrs[i]).reshape(
                NCORES, *out_avals[i].shape)[c]
             for i, name in enumerate(out_names)}
            for c in range(NCORES)
        ]

    return run


_RUNNERS = {}


def kernel(**inputs):
    nc, in_maps, meta = prepare_and_build(inputs)
    key = id(nc)
    if key not in _RUNNERS:
        _RUNNERS[key] = _make_runner(nc)
    last_err = None
    for attempt in range(3):
        try:
            results = _RUNNERS[key](in_maps)
            return unshard(results, meta)
        except Exception as e:  # transient device / axon failures
            last_err = e
            import time as _time
            _time.sleep(5)
            _RUNNERS[key] = _make_runner(nc)
    # final fallback: the stock helper
    try:
        res = run_bass_kernel_spmd(nc, in_maps, core_ids=list(range(NCORES)))
        return unshard(res.results, meta)
    except Exception:
        raise last_err

